# revision 1
# baseline (speedup 1.0000x reference)
"""Trainium2 Bass kernel for nn_AlignmentLayer (Kabsch alignment of L frames).

Strategy (pure data parallel over 8 NeuronCores, L/8 = 8192 frames per core):

Host-side (numpy, cheap layout work only):
  - ref_c = ref_x - mean(ref_x); gather xg = x[:, align_idx, :]  (align_idx is
    a host-known constant input, so the gather folds into data layout).
  - xgT: gathered atoms pre-transposed to [256(pad of 192), L] so phase 1 needs
    zero on-chip transposes.
  - x_sep: x in component-major layout [L, 3, 256] so phase-2 tensor ops are
    contiguous; output produced component-major and un-permuted on host.
  - W: [256, 12] weights mapping gathered rows to the 9 entries of
    A = xg^T @ ref_c and the 3 entries of the centroid x_c.

Device (per core), three phases:
  1. PE matmuls: per 128-frame tile, E[128, 12] = xgT_tile^T @ W (two K=128
     chunks accumulated in PSUM), evacuated by ScalarE into E_all.
  2. Math (DVE + ScalarE, batched [128, 64] ops): SVD-free Kabsch rotation.
     S = A^T A; lambda1 via trigonometric cubic (arctan+sin);
     v1 = best cross product of rows of (S - lambda1 I); (v2, v3) from a
     deflated 2x2 eigenproblem in the complement; u_i = normalize(A v_i);
     u3 = u1 x u2; R = sum u_i v_i^T (reflection handled automatically by
     det=+1 frames); tneg = -x_c R.  All sqrt Newton-polished.
  3. Apply: out_b = (x_0 R0b + tneg_b) + x_1 R1b + x_2 R2b via per-partition
     scalar MADs (ACT activation + DVE/GPSIMD scalar_tensor_tensor).
"""

import numpy as np

L_FULL = 65536
N_INP = 256
N_ALIGN = 64
N_CORES = 8
LS = L_FULL // N_CORES          # frames per core
NT = LS // 128                  # 128-frame tiles per core (64)
F32 = np.float32

_RUNNER = None


# ----------------------------------------------------------------------------
# Math IR: record ops on virtual registers, then emit with linear-scan slot
# assignment into one scratch tensor (plain RAW/WAR deps; no pool cap-gate).
# ----------------------------------------------------------------------------

class _VR(int):
    """Virtual register id."""


class _MathIR:
    def __init__(self, alu):
        self.A_ = alu
        self.ops = []           # (kind, out, ins, extra)
        self.n = 0

    def _rec(self, kind, ins, extra=None, out=None):
        if out is None:
            out = _VR(self.n)
            self.n += 1
        self.ops.append((kind, out, list(ins), extra))
        return out

    def tt(self, op, a, b, out=None):
        return self._rec("tt", [a, b], op, out)

    def mul(self, a, b, out=None):
        return self.tt(self.A_.mult, a, b, out)

    def add(self, a, b, out=None):
        return self.tt(self.A_.add, a, b, out)

    def sub(self, a, b, out=None):
        return self.tt(self.A_.subtract, a, b, out)

    def ts(self, a, s1, op0, s2=None, op1=None, out=None):
        return self._rec("ts", [a], (float(s1), op0,
                                     None if s2 is None else float(s2), op1), out)

    def act(self, fn, a, scale=1.0, bias=None, out=None):
        return self._rec("act", [a], (fn, scale, bias), out)

    def recip(self, a, out=None):
        return self._rec("recip", [a], None, out)

    def rsqrt_pol(self, nval):
        """1/sqrt(n), one Newton step (ACT Sqrt is low-precision)."""
        from concourse import mybir
        AF = mybir.ActivationFunctionType
        s0 = self.act(AF.Sqrt, nval)
        y = self.recip(s0)
        y2 = self.mul(y, y)
        ny2 = self.mul(nval, y2)
        h = self.ts(ny2, -0.5, self.A_.mult, 1.5, self.A_.add)
        return self.mul(y, h)

    def dot3(self, ax, ay, az, bx, by, bz):
        t1 = self.mul(ax, bx)
        t2 = self.mul(ay, by)
        s = self.add(t1, t2)
        t3 = self.mul(az, bz)
        return self.add(s, t3)

    def cross3(self, a, b):
        cx = self.sub(self.mul(a[1], b[2]), self.mul(a[2], b[1]))
        cy = self.sub(self.mul(a[2], b[0]), self.mul(a[0], b[2]))
        cz = self.sub(self.mul(a[0], b[1]), self.mul(a[1], b[0]))
        return [cx, cy, cz]

    def blend3(self, m, a, b):
        out = []
        for i in range(3):
            d = self.sub(a[i], b[i])
            out.append(self.add(b[i], self.mul(m, d)))
        return out


def _emit_math(nc, ir, ms_ap, C, n_slots):
    """Emit recorded IR. Vreg v lives in ms_ap[:, slot*C:(slot+1)*C]."""
    last_use = {}
    for i, (kind, out, ins, extra) in enumerate(ir.ops):
        for v in ins:
            if isinstance(v, _VR):
                last_use[int(v)] = i
    free = list(range(n_slots - 1, -1, -1))
    slot_of = {}

    def ap_of(v):
        if isinstance(v, _VR):
            s = slot_of[int(v)]
            return ms_ap[:, s * C:(s + 1) * C]
        return v  # external AP

    for i, (kind, out, ins, extra) in enumerate(ir.ops):
        if isinstance(out, _VR):
            slot = free.pop()
            slot_of[int(out)] = slot
            out_ap = ms_ap[:, slot * C:(slot + 1) * C]
        else:
            out_ap = out
        in_aps = [ap_of(v) for v in ins]
        if kind == "tt":
            nc.vector.tensor_tensor(out_ap, in_aps[0], in_aps[1], extra)
        elif kind == "ts":
            s1, op0, s2, op1 = extra
            if s2 is None:
                nc.vector.tensor_scalar(out_ap, in_aps[0], s1, None, op0)
            else:
                nc.vector.tensor_scalar(out_ap, in_aps[0], s1, s2, op0, op1)
        elif kind == "act":
            fn, scale, bias = extra
            if bias is None:
                nc.scalar.activation(out_ap, in_aps[0], fn, scale=scale)
            else:
                nc.scalar.activation(out_ap, in_aps[0], fn, scale=scale, bias=bias)
        elif kind == "recip":
            nc.vector.reciprocal(out_ap, in_aps[0])
        else:
            raise ValueError(kind)
        # free operands at their last use (dedupe: an op may use a vreg twice)
        for vi in {int(v) for v in ins if isinstance(v, _VR)}:
            if last_use.get(vi) == i:
                free.append(slot_of[vi])
        # a value never read would leak its slot; assert instead
        assert free or i == len(ir.ops) - 1, "scratch slots exhausted"


def _record_math(ir, Ev, Rv, pi3_ap):
    """Record the whole rotation math on the IR. Ev/Rv are [128, 12, C] views
    (strided entry slices); pi3_ap is a [128,1] const with pi/3."""
    from concourse import mybir
    AF = mybir.ActivationFunctionType
    A_ = ir.A_

    Ae = [[Ev[:, 3 * a + b, :] for b in range(3)] for a in range(3)]
    me = [Ev[:, 9 + a, :] for a in range(3)]

    # S = A^T A (6 unique entries)
    Smat = {}
    for bi in range(3):
        for ci in range(bi, 3):
            Smat[(bi, ci)] = ir.dot3(Ae[0][bi], Ae[1][bi], Ae[2][bi],
                                     Ae[0][ci], Ae[1][ci], Ae[2][ci])

    def S(i, j):
        return Smat[(min(i, j), max(i, j))]

    q = ir.ts(ir.add(ir.add(S(0, 0), S(1, 1)), S(2, 2)), 1.0 / 3.0, A_.mult)
    P00 = ir.sub(S(0, 0), q)
    P11 = ir.sub(S(1, 1), q)
    P22 = ir.sub(S(2, 2), q)
    sq01 = ir.mul(S(0, 1), S(0, 1))
    sq02 = ir.mul(S(0, 2), S(0, 2))
    sq12 = ir.mul(S(1, 2), S(1, 2))
    diagsq = ir.add(ir.add(ir.mul(P00, P00), ir.mul(P11, P11)), ir.mul(P22, P22))
    offsq = ir.add(ir.add(sq01, sq02), sq12)
    p2v = ir.add(diagsq, ir.ts(offsq, 2.0, A_.mult))
    p2c = ir.ts(ir.ts(p2v, 1.0 / 6.0, A_.mult), 1e-30, A_.max)
    pinv = ir.rsqrt_pol(p2c)
    pval = ir.mul(p2c, pinv)

    c0 = ir.sub(ir.mul(P11, P22), sq12)
    c1c = ir.sub(ir.mul(S(0, 1), P22), ir.mul(S(1, 2), S(0, 2)))
    c2c = ir.sub(ir.mul(S(0, 1), S(1, 2)), ir.mul(P11, S(0, 2)))
    detB = ir.add(ir.sub(ir.mul(P00, c0), ir.mul(S(0, 1), c1c)),
                  ir.mul(S(0, 2), c2c))
    pinv3 = ir.mul(ir.mul(pinv, pinv), pinv)
    rr = ir.ts(ir.mul(detB, pinv3), 0.5, A_.mult, 0.9999995, A_.min)
    rr = ir.ts(rr, -0.9999995, A_.max)

    omr = ir.ts(ir.mul(rr, rr), -1.0, A_.mult, 1.0, A_.add)
    rs = ir.rsqrt_pol(omr)
    uu = ir.mul(rr, rs)
    # arctan(u) with range reduction — ACT Arctan domain is [-pi/2, pi/2]:
    # |u|<=1: a = arctan(|u|); |u|>1: pi/2 - arctan(1/|u|); then apply sign.
    au = ir.tt(A_.max, uu, ir.ts(uu, -1.0, A_.mult))      # |u|
    inv = ir.recip(ir.ts(au, 1e-30, A_.max))
    z = ir.tt(A_.min, au, inv)
    az = ir.act(AF.Arctan, z)
    dz = ir.ts(az, -1.0, A_.mult, float(np.pi / 2), A_.add)
    mge = ir.ts(au, 1.0, A_.is_ge)                        # |u| >= 1
    mle = ir.ts(mge, -1.0, A_.mult, 1.0, A_.add)          # 1 - that
    res_abs = ir.add(dz, ir.mul(mle, ir.sub(az, dz)))
    sgn_u = ir.ts(ir.ts(uu, 0.0, A_.is_ge), 2.0, A_.mult, -1.0, A_.add)
    at = ir.mul(res_abs, sgn_u)
    c1t = ir.act(AF.Sin, at, scale=1.0 / 3.0, bias=pi3_ap)
    lam1 = ir.add(q, ir.ts(ir.mul(pval, c1t), 2.0, A_.mult))

    # v1 = best cross of rows of (S - lam1 I)
    D0 = ir.sub(S(0, 0), lam1)
    D1 = ir.sub(S(1, 1), lam1)
    D2 = ir.sub(S(2, 2), lam1)
    rows = [
        [D0, S(0, 1), S(0, 2)],
        [S(0, 1), D1, S(1, 2)],
        [S(0, 2), S(1, 2), D2],
    ]
    best, bn = None, None
    for (i, j) in [(0, 1), (0, 2), (1, 2)]:
        c = ir.cross3(rows[i], rows[j])
        n = ir.dot3(c[0], c[1], c[2], c[0], c[1], c[2])
        if best is None:
            best, bn = c, n
        else:
            m = ir.tt(A_.is_gt, n, bn)
            best = ir.blend3(m, c, best)
            bn = ir.add(bn, ir.mul(m, ir.sub(n, bn)))
    inv = ir.rsqrt_pol(ir.ts(bn, 1e-37, A_.max))
    v1 = [ir.mul(best[0], inv), ir.mul(best[1], inv), ir.mul(best[2], inv)]

    # w2 = best of cross(v1, e_k) (candidates have a zero component)
    zero = ir.ts(v1[0], 0.0, A_.mult)
    nv1 = [ir.ts(v1[i], -1.0, A_.mult) for i in range(3)]
    sqv = [ir.mul(v1[i], v1[i]) for i in range(3)]
    cands = [
        [zero, v1[2], nv1[1]],
        [nv1[2], zero, v1[0]],
        [v1[1], nv1[0], zero],
    ]
    cns = [ir.add(sqv[1], sqv[2]), ir.add(sqv[0], sqv[2]), ir.add(sqv[0], sqv[1])]
    w2b, wn = cands[0], cns[0]
    for k in range(1, 3):
        m = ir.tt(A_.is_gt, cns[k], wn)
        w2b = ir.blend3(m, cands[k], w2b)
        wn = ir.add(wn, ir.mul(m, ir.sub(cns[k], wn)))
    winv = ir.rsqrt_pol(ir.ts(wn, 1e-37, A_.max))
    w2 = [ir.mul(w2b[i], winv) for i in range(3)]
    w3 = ir.cross3(v1, w2)

    def Svec(v):
        return [ir.dot3(S(bi, 0), S(bi, 1), S(bi, 2), v[0], v[1], v[2])
                for bi in range(3)]

    Sw2 = Svec(w2)
    Sw3 = Svec(w3)
    a2x = ir.dot3(w2[0], w2[1], w2[2], Sw2[0], Sw2[1], Sw2[2])
    b2x = ir.dot3(w2[0], w2[1], w2[2], Sw3[0], Sw3[1], Sw3[2])
    c2x = ir.dot3(w3[0], w3[1], w3[2], Sw3[0], Sw3[1], Sw3[2])

    half = ir.ts(ir.sub(a2x, c2x), 0.5, A_.mult)
    mpos = ir.ts(half, 0.0, A_.is_ge)
    sgn = ir.ts(mpos, 2.0, A_.mult, -1.0, A_.add)
    habs = ir.mul(sgn, half)
    rad2 = ir.ts(ir.add(ir.mul(half, half), ir.mul(b2x, b2x)), 1e-37, A_.max)
    radi = ir.rsqrt_pol(rad2)
    rad = ir.mul(rad2, radi)
    pos = ir.ts(ir.add(habs, rad), 1e-37, A_.max)
    tq = ir.mul(ir.mul(b2x, ir.recip(pos)), sgn)
    c2i = ir.rsqrt_pol(ir.ts(ir.mul(tq, tq), 1.0, A_.add))
    s2i = ir.mul(tq, c2i)
    tb = ir.mul(tq, b2x)
    lamA = ir.add(a2x, tb)
    lamB = ir.sub(c2x, tb)
    mAB = ir.tt(A_.is_ge, lamA, lamB)
    vA = [ir.add(ir.mul(c2i, w2[i]), ir.mul(s2i, w3[i])) for i in range(3)]
    vB = [ir.sub(ir.mul(c2i, w3[i]), ir.mul(s2i, w2[i])) for i in range(3)]
    v2 = ir.blend3(mAB, vA, vB)
    v3 = ir.cross3(v1, v2)

    def Avec(v):
        return [ir.dot3(Ae[ai][0], Ae[ai][1], Ae[ai][2], v[0], v[1], v[2])
                for ai in range(3)]

    b1 = Avec(v1)
    n1 = ir.dot3(b1[0], b1[1], b1[2], b1[0], b1[1], b1[2])
    i1 = ir.rsqrt_pol(ir.ts(n1, 1e-37, A_.max))
    u1 = [ir.mul(b1[i], i1) for i in range(3)]

    b2v = Avec(v2)
    dd = ir.dot3(u1[0], u1[1], u1[2], b2v[0], b2v[1], b2v[2])
    b2o = [ir.sub(b2v[i], ir.mul(dd, u1[i])) for i in range(3)]
    n2 = ir.dot3(b2o[0], b2o[1], b2o[2], b2o[0], b2o[1], b2o[2])
    i2 = ir.rsqrt_pol(ir.ts(n2, 1e-37, A_.max))
    u2 = [ir.mul(b2o[i], i2) for i in range(3)]

    u3 = ir.cross3(u1, u2)

    us = [u1, u2, u3]
    vs = [v1, v2, v3]
    for ai in range(3):
        for bi in range(3):
            t1 = ir.mul(us[0][ai], vs[0][bi])
            t2 = ir.mul(us[1][ai], vs[1][bi])
            sgm = ir.add(t1, t2)
            t3 = ir.mul(us[2][ai], vs[2][bi])
            ir.add(sgm, t3, out=Rv[:, 3 * ai + bi, :])

    mn = [ir.ts(me[i], -1.0, A_.mult) for i in range(3)]
    for bi in range(3):
        t1 = ir.mul(mn[0], Rv[:, bi, :])
        t2 = ir.mul(mn[1], Rv[:, 3 + bi, :])
        sgm = ir.add(t1, t2)
        t3 = ir.mul(mn[2], Rv[:, 6 + bi, :])
        ir.add(sgm, t3, out=Rv[:, 9 + bi, :])


# ----------------------------------------------------------------------------
# Bass program
# ----------------------------------------------------------------------------

def _split_multiwait(nc):
    """This walrus build encodes at most ONE semaphore wait per instruction,
    but Tile emits several. Split extras into standalone EventSemaphore
    (pure wait) instructions on the same engine, immediately before."""
    from concourse import mybir
    import bass_rust

    n_split = 0
    for fn in nc.m.functions:
        for blk in fn.blocks:
            new = []
            for ins in blk.instructions:
                si = ins.sync_info
                if si is not None and si.on_wait is not None and len(si.on_wait) > 1:
                    waits = list(si.on_wait)
                    for k, w in enumerate(waits[:-1]):
                        new.append(mybir.InstEventSemaphore(
                            name=f"{ins.name}-w{k}",
                            engine=ins.engine,
                            sync_info=bass_rust.SyncInfo(
                                on_wait=[w], on_update=[]),
                        ))
                        n_split += 1
                    ins.sync_info = bass_rust.SyncInfo(
                        on_wait=[waits[-1]],
                        on_update=list(si.on_update or []))
                new.append(ins)
            blk.instructions = new
    return n_split


def _build_program(ls=LS, n_slots=56, split_waits=True, repeat=1,
                   math_repeat=1, p3_repeat=1, math_chunks=2):
    import concourse.bass as bass
    import concourse.tile as tile
    from concourse import mybir

    f32 = mybir.dt.float32
    A_ = mybir.AluOpType
    AF = mybir.ActivationFunctionType

    nt = ls // 128
    C = nt

    nc = bass.Bass("TRN2", target_bir_lowering=False, debug=False)

    xgt_d = nc.dram_tensor("xgt", [256, ls], f32, kind="ExternalInput").ap()
    xsep_d = nc.dram_tensor("xsep", [ls, 768], f32, kind="ExternalInput").ap()
    w_d = nc.dram_tensor("wm", [256, 12], f32, kind="ExternalInput").ap()
    out_d = nc.dram_tensor("out", [ls, 768], f32, kind="ExternalOutput").ap()

    with tile.TileContext(nc) as tc:
        with (
            tc.tile_pool(name="wp", bufs=1) as wp,
            tc.tile_pool(name="gp_", bufs=2) as gpool,
            tc.tile_pool(name="ep", bufs=1) as ep,
            tc.tile_pool(name="ps", bufs=7, space="PSUM") as psp,
            tc.tile_pool(name="ps2", bufs=1, space="PSUM") as pss,
            tc.tile_pool(name="xp", bufs=6) as xp,
            tc.tile_pool(name="p2", bufs=3) as p2p,
            tc.tile_pool(name="op_", bufs=3) as opool,
        ):
            # ---------------- constants / weights ----------------
            w0 = wp.tile([128, 12], f32, tag="w0")
            w1 = wp.tile([128, 12], f32, tag="w1")
            nc.sync.dma_start(w0[:], w_d[0:128, :])
            nc.sync.dma_start(w1[:], w_d[128:256, :])

            E = ep.tile([128, nt * 12], f32, tag="E")
            R = ep.tile([128, nt * 12], f32, tag="R")
            MS = ep.tile([128, n_slots * C], f32, tag="MS")
            pi3 = ep.tile([128, 1], f32, tag="pi3")
            nc.gpsimd.memset(pi3[:], float(np.pi / 3))
            Ev = E[:].rearrange("p (g e) -> p e g", e=12)
            Rv = R[:].rearrange("p (g e) -> p e g", e=12)

            for _rep in range(repeat):
                # ---------------- phase 1: E = xgT^T @ W ----------------
                # The Matmult ISA slot encodes at most ONE semaphore wait, so each
                # real matmul must need at most one fresh semaphore. Dummy PE
                # matmuls "absorb" each DMA's semaphore into the PE's observed
                # clock first (engine-internal ordering then needs no sems).
                ps_scr = pss.tile([128, 12], f32, tag="scr")
                nc.tensor.matmul(ps_scr[0:12, 0:12], w0[:], w0[:],
                                 start=True, stop=True)
                nc.tensor.matmul(ps_scr[0:12, 0:12], w1[:], w1[:],
                                 start=True, stop=True)
                n_slab = nt // 16
                for s in range(n_slab):
                    sl0 = gpool.tile([128, 2048], f32, tag="g0")
                    sl1 = gpool.tile([128, 2048], f32, tag="g1")
                    nc.sync.dma_start(sl0[:], xgt_d[0:128, s * 2048:(s + 1) * 2048])
                    nc.sync.dma_start(sl1[:], xgt_d[128:256, s * 2048:(s + 1) * 2048])
                    nc.tensor.matmul(ps_scr[0:12, 0:12], sl0[:, 0:12], sl0[:, 0:12],
                                     start=True, stop=True)
                    nc.tensor.matmul(ps_scr[0:12, 0:12], sl1[:, 0:12], sl1[:, 0:12],
                                     start=True, stop=True)
                    for g in range(16):
                        gg = s * 16 + g
                        ps = psp.tile([128, 12], f32, tag="eps")
                        nc.tensor.matmul(ps[:], sl0[:, g * 128:(g + 1) * 128], w0[:],
                                         start=True, stop=False)
                        nc.tensor.matmul(ps[:], sl1[:, g * 128:(g + 1) * 128], w1[:],
                                         start=False, stop=True)
                        nc.scalar.copy(E[:, gg * 12:(gg + 1) * 12], ps[:])

                # ---------------- phases 2+3, chunked for pipelining ------
                ct = nt // math_chunks       # tiles per chunk
                for h in range(math_chunks):
                  Ev_h = E[:, h * ct * 12:(h + 1) * ct * 12].rearrange(
                      "p (g e) -> p e g", e=12)
                  Rv_h = R[:, h * ct * 12:(h + 1) * ct * 12].rearrange(
                      "p (g e) -> p e g", e=12)
                  for _mrep in range(math_repeat):
                    ir = _MathIR(A_)
                    _record_math(ir, Ev_h, Rv_h, pi3[:])
                    _emit_math(nc, ir, MS[:], ct, n_slots)

                  for _prep in range(p3_repeat):
                    # ---------------- phase 3: apply ----------------
                    n_grp = ct // 4
                    for grp in range(h * n_grp, (h + 1) * n_grp):
                        xq = xp.tile([128, 4 * 768], f32, tag="xq")
                        src = xsep_d[grp * 512:(grp + 1) * 512, :].rearrange(
                            "(g p) c -> p g c", p=128)
                        nc.sync.dma_start(xq[:].rearrange("p (g c) -> p g c", c=768), src)
                        for t in range(4):
                            gg = grp * 4 + t
                            base = t * 768
                            if t % 2 == 0:
                                ot = opool.tile([128, 2 * 768], f32, tag="ot")
                            obase = (t % 2) * 768
                            # u = x0*R0b + tneg_b (ACT MAD); m1 = x1*R1b + u (DVE
                            # fused, into a3 block); w = x2*R2b (DVE/ACT split);
                            # out_tile = a3 + w3 in ONE GPSIMD add over all three
                            # components (POOL per-instruction dispatch is costly
                            # and it cannot run AP-scalar ops in this toolchain).
                            a3 = p2p.tile([128, 768], f32, tag="a3")
                            w3 = p2p.tile([128, 768], f32, tag="w3")
                            for bi in range(3):
                                rcol0 = R[:, gg * 12 + bi: gg * 12 + bi + 1]
                                rcol1 = R[:, gg * 12 + 3 + bi: gg * 12 + 3 + bi + 1]
                                rcol2 = R[:, gg * 12 + 6 + bi: gg * 12 + 6 + bi + 1]
                                tncol = R[:, gg * 12 + 9 + bi: gg * 12 + 9 + bi + 1]
                                x0 = xq[:, base:base + 256]
                                x1 = xq[:, base + 256:base + 512]
                                x2 = xq[:, base + 512:base + 768]
                                u_t = p2p.tile([128, 256], f32, tag=f"u{bi}")
                                nc.scalar.activation(u_t[:], x0, AF.Identity,
                                                     bias=tncol, scale=rcol0)
                                nc.vector.scalar_tensor_tensor(
                                    a3[:, bi * 256:(bi + 1) * 256],
                                    x1, rcol1, u_t[:], A_.mult, A_.add)
                                wslice = w3[:, bi * 256:(bi + 1) * 256]
                                if (gg + bi) % 2 == 0:
                                    nc.vector.tensor_scalar(
                                        wslice, x2, rcol2, None, A_.mult)
                                else:
                                    nc.scalar.activation(wslice, x2, AF.Identity,
                                                         scale=rcol2)
                            nc.gpsimd.tensor_tensor(
                                ot[:, obase:obase + 768], a3[:], w3[:], A_.add)
                            if t % 2 == 1:
                                dst = out_d[(gg - 1) * 128:(gg + 1) * 128, :].rearrange(
                                    "(g p) c -> p g c", p=128)
                                nc.sync.dma_start(dst, ot[:].rearrange(
                                    "p (g c) -> p g c", c=768))

    if split_waits:
        _split_multiwait(nc)
    return nc


# ----------------------------------------------------------------------------
# Host-side preparation
# ----------------------------------------------------------------------------

def _prep_inputs(x, ref_x, align_idx):
    x = np.asarray(x, dtype=F32)
    ref_x = np.asarray(ref_x)
    idx = np.asarray(align_idx).astype(np.int64)
    L = x.shape[0]

    ref64 = ref_x.astype(np.float64)
    ref_c = (ref64 - ref64.mean(0)).astype(F32)        # [64, 3]

    xg = x[:, idx, :]                                   # [L, 64, 3]
    xgt = np.zeros((256, L), dtype=F32)
    xgt[:192] = np.ascontiguousarray(xg.reshape(L, 192).T)

    xsep = np.ascontiguousarray(x.transpose(0, 2, 1)).reshape(L, 768)

    W = np.zeros((256, 12), dtype=F32)
    for a in range(3):
        rows = 3 * np.arange(N_ALIGN) + a
        for b in range(3):
            W[rows, 3 * a + b] = ref_c[:, b]
        W[rows, 9 + a] = F32(1.0 / N_ALIGN)
    return xgt, xsep, W


# ----------------------------------------------------------------------------
# Runner: jit once, reuse
# ----------------------------------------------------------------------------

class _Runner:
    def __init__(self, repeat=1):
        import jax

        self.jax = jax
        self.nc = _build_program(LS, repeat=repeat)
        self._build_exec()

    def _build_exec(self):
        import jax
        from jax.sharding import Mesh, PartitionSpec
        from jax.experimental.shard_map import shard_map
        from concourse import mybir
        from concourse.bass2jax import (_bass_exec_p, install_neuronx_cc_hook,
                                        partition_id_tensor)

        install_neuronx_cc_hook()
        # surface compile-hook exceptions (PJRT swallows them)
        try:
            import libneuronxla
            import traceback
            if not getattr(libneuronxla, "_ant_logged_cc", False):
                _orig_cc = libneuronxla.neuronx_cc

                def _logged_cc(*a, **k):
                    try:
                        return _orig_cc(*a, **k)
                    except BaseException:
                        traceback.print_exc()
                        raise

                libneuronxla.neuronx_cc = _logged_cc
                libneuronxla._ant_logged_cc = True
        except ImportError:
            pass
        nc = self.nc

        part_name = (nc.partition_id_tensor.name
                     if nc.partition_id_tensor else None)
        in_names, out_names, out_avals = [], [], []
        for alloc in nc.m.functions[0].allocations:
            if not isinstance(alloc, mybir.MemoryLocationSet):
                continue
            name = alloc.memorylocations[0].name
            if alloc.kind == "ExternalInput":
                if name != part_name:
                    in_names.append(name)
            elif alloc.kind == "ExternalOutput":
                shape = tuple(alloc.tensor_shape)
                dtype = mybir.dt.np(alloc.dtype)
                out_names.append(name)
                out_avals.append(jax.core.ShapedArray(shape, dtype))
        self.in_names = list(in_names)
        self.out_names = list(out_names)
        n_params = len(in_names)
        all_names = in_names + out_names
        if part_name is not None:
            all_names = all_names + [part_name]

        def _body(*args):
            operands = list(args)
            if part_name is not None:
                operands.append(partition_id_tensor())
            outs = _bass_exec_p.bind(
                *operands,
                out_avals=tuple(out_avals),
                in_names=tuple(all_names),
                out_names=tuple(out_names),
                lowering_input_output_aliases=(),
                sim_require_finite=True,
                sim_require_nnan=True,
                nc=nc,
            )
            return tuple(outs)

        devices = jax.devices()[:N_CORES]
        mesh = Mesh(np.asarray(devices), ("core",))
        n_outs = len(out_names)
        in_specs = (PartitionSpec("core"),) * (n_params + n_outs)
        out_specs = (PartitionSpec("core"),) * n_outs
        self._fn = jax.jit(
            shard_map(_body, mesh=mesh, in_specs=in_specs,
                      out_specs=out_specs, check_rep=False),
            keep_unused=True,
        )
        self._zeros = [
            np.zeros((N_CORES * av.shape[0], *av.shape[1:]), av.dtype)
            for av in out_avals
        ]

    def stage(self, x, ref_x, align_idx):
        xgt, xsep, W = _prep_inputs(x, ref_x, align_idx)
        per_name = {
            "xgt": np.concatenate(
                [xgt[:, c * LS:(c + 1) * LS] for c in range(N_CORES)], axis=0),
            "xsep": xsep,
            "wm": np.concatenate([W] * N_CORES, axis=0),
        }
        args = [per_name[n] for n in self.in_names] + list(self._zeros)
        return [self.jax.device_put(a) for a in args]

    def run_staged(self, staged):
        return self._fn(*staged)

    def run(self, x, ref_x, align_idx):
        staged = self.stage(x, ref_x, align_idx)
        outs = self.run_staged(staged)
        out = np.asarray(outs[self.out_names.index("out")])
        L = out.shape[0]
        return np.ascontiguousarray(
            out.reshape(L, 3, N_INP).transpose(0, 2, 1))


def _get_runner():
    global _RUNNER
    if _RUNNER is None:
        _RUNNER = _Runner()
    return _RUNNER


def kernel(x, ref_x, align_idx):
    runner = _get_runner()
    return runner.run(x, ref_x, align_idx).astype(np.float32)


if __name__ == "__main__":
    nc = _build_program(LS)
    print("built ok")



# revision 5
# speedup vs baseline: 1.4882x; 1.4882x over previous
"""Trainium2 Bass kernel for nn_AlignmentLayer (Kabsch alignment of L frames).

Strategy (pure data parallel over 8 NeuronCores, L/8 = 8192 frames per core):

Host-side (numpy, cheap layout work only):
  - ref_c = ref_x - mean(ref_x); gather xg = x[:, align_idx, :]  (align_idx is
    a host-known constant input, so the gather folds into data layout).
  - xgT: gathered atoms pre-transposed to [256(pad of 192), L] so phase 1 needs
    zero on-chip transposes.
  - x_sep: x in component-major layout [L, 3, 256] so phase-2 tensor ops are
    contiguous; output produced component-major and un-permuted on host.
  - W: [256, 12] weights mapping gathered rows to the 9 entries of
    A = xg^T @ ref_c and the 3 entries of the centroid x_c.

Device (per core), three phases:
  1. PE matmuls: per 128-frame tile, E[128, 12] = xgT_tile^T @ W (two K=128
     chunks accumulated in PSUM), evacuated by ScalarE into E_all.
  2. Math (DVE + ScalarE, batched [128, 64] ops): SVD-free Kabsch rotation.
     S = A^T A; lambda1 via trigonometric cubic (arctan+sin);
     v1 = best cross product of rows of (S - lambda1 I); (v2, v3) from a
     deflated 2x2 eigenproblem in the complement; u_i = normalize(A v_i);
     u3 = u1 x u2; R = sum u_i v_i^T (reflection handled automatically by
     det=+1 frames); tneg = -x_c R.  All sqrt Newton-polished.
  3. Apply: out_b = (x_0 R0b + tneg_b) + x_1 R1b + x_2 R2b via per-partition
     scalar MADs (ACT activation + DVE/GPSIMD scalar_tensor_tensor).
"""

import numpy as np

L_FULL = 65536
N_INP = 256
N_ALIGN = 64
N_CORES = 8
LS = L_FULL // N_CORES          # frames per core
NT = LS // 128                  # 128-frame tiles per core (64)
F32 = np.float32

_RUNNER = None


# ----------------------------------------------------------------------------
# Math IR: record ops on virtual registers, then emit with linear-scan slot
# assignment into one scratch tensor (plain RAW/WAR deps; no pool cap-gate).
# ----------------------------------------------------------------------------

class _VR(int):
    """Virtual register id."""


class _MathIR:
    def __init__(self, alu):
        self.A_ = alu
        self.ops = []           # (kind, out, ins, extra)
        self.n = 0

    def _rec(self, kind, ins, extra=None, out=None):
        if out is None:
            out = _VR(self.n)
            self.n += 1
        self.ops.append((kind, out, list(ins), extra))
        return out

    def tt(self, op, a, b, out=None):
        return self._rec("tt", [a, b], op, out)

    def mul(self, a, b, out=None):
        return self.tt(self.A_.mult, a, b, out)

    def add(self, a, b, out=None):
        return self.tt(self.A_.add, a, b, out)

    def sub(self, a, b, out=None):
        return self.tt(self.A_.subtract, a, b, out)

    def ts(self, a, s1, op0, s2=None, op1=None, out=None):
        return self._rec("ts", [a], (float(s1), op0,
                                     None if s2 is None else float(s2), op1), out)

    def act(self, fn, a, scale=1.0, bias=None, out=None):
        return self._rec("act", [a], (fn, scale, bias), out)

    def recip(self, a, out=None):
        return self._rec("recip", [a], None, out)

    def rsqrt_pol(self, nval):
        """1/sqrt(n), one Newton step (ACT Sqrt is low-precision)."""
        from concourse import mybir
        AF = mybir.ActivationFunctionType
        s0 = self.act(AF.Sqrt, nval)
        y = self.recip(s0)
        y2 = self.mul(y, y)
        ny2 = self.mul(nval, y2)
        h = self.ts(ny2, -0.5, self.A_.mult, 1.5, self.A_.add)
        return self.mul(y, h)

    def dot3(self, ax, ay, az, bx, by, bz):
        t1 = self.mul(ax, bx)
        t2 = self.mul(ay, by)
        s = self.add(t1, t2)
        t3 = self.mul(az, bz)
        return self.add(s, t3)

    def cross3(self, a, b):
        cx = self.sub(self.mul(a[1], b[2]), self.mul(a[2], b[1]))
        cy = self.sub(self.mul(a[2], b[0]), self.mul(a[0], b[2]))
        cz = self.sub(self.mul(a[0], b[1]), self.mul(a[1], b[0]))
        return [cx, cy, cz]

    def blend3(self, m, a, b):
        out = []
        for i in range(3):
            d = self.sub(a[i], b[i])
            out.append(self.add(b[i], self.mul(m, d)))
        return out


def _emit_math(nc, ir, ms_ap, C, n_slots):
    """Emit recorded IR. Vreg v lives in ms_ap[:, slot*C:(slot+1)*C]."""
    last_use = {}
    for i, (kind, out, ins, extra) in enumerate(ir.ops):
        for v in ins:
            if isinstance(v, _VR):
                last_use[int(v)] = i
    free = list(range(n_slots - 1, -1, -1))
    slot_of = {}

    def ap_of(v):
        if isinstance(v, _VR):
            s = slot_of[int(v)]
            return ms_ap[:, s * C:(s + 1) * C]
        return v  # external AP

    for i, (kind, out, ins, extra) in enumerate(ir.ops):
        if isinstance(out, _VR):
            slot = free.pop()
            slot_of[int(out)] = slot
            out_ap = ms_ap[:, slot * C:(slot + 1) * C]
        else:
            out_ap = out
        in_aps = [ap_of(v) for v in ins]
        if kind == "tt":
            nc.vector.tensor_tensor(out_ap, in_aps[0], in_aps[1], extra)
        elif kind == "ts":
            s1, op0, s2, op1 = extra
            if s2 is None:
                nc.vector.tensor_scalar(out_ap, in_aps[0], s1, None, op0)
            else:
                nc.vector.tensor_scalar(out_ap, in_aps[0], s1, s2, op0, op1)
        elif kind == "act":
            fn, scale, bias = extra
            if bias is None:
                nc.scalar.activation(out_ap, in_aps[0], fn, scale=scale)
            else:
                nc.scalar.activation(out_ap, in_aps[0], fn, scale=scale, bias=bias)
        elif kind == "recip":
            nc.vector.reciprocal(out_ap, in_aps[0])
        else:
            raise ValueError(kind)
        # free operands at their last use (dedupe: an op may use a vreg twice)
        for vi in {int(v) for v in ins if isinstance(v, _VR)}:
            if last_use.get(vi) == i:
                free.append(slot_of[vi])
        # a value never read would leak its slot; assert instead
        assert free or i == len(ir.ops) - 1, "scratch slots exhausted"


def _record_math(ir, Ev, Rv, pi3_ap):
    """Record the whole rotation math on the IR. Ev/Rv are [128, 12, C] views
    (strided entry slices); pi3_ap is a [128,1] const with pi/3."""
    from concourse import mybir
    AF = mybir.ActivationFunctionType
    A_ = ir.A_

    Ae = [[Ev[:, 3 * a + b, :] for b in range(3)] for a in range(3)]
    me = [Ev[:, 9 + a, :] for a in range(3)]

    # S = A^T A (6 unique entries)
    Smat = {}
    for bi in range(3):
        for ci in range(bi, 3):
            Smat[(bi, ci)] = ir.dot3(Ae[0][bi], Ae[1][bi], Ae[2][bi],
                                     Ae[0][ci], Ae[1][ci], Ae[2][ci])

    def S(i, j):
        return Smat[(min(i, j), max(i, j))]

    q = ir.ts(ir.add(ir.add(S(0, 0), S(1, 1)), S(2, 2)), 1.0 / 3.0, A_.mult)
    P00 = ir.sub(S(0, 0), q)
    P11 = ir.sub(S(1, 1), q)
    P22 = ir.sub(S(2, 2), q)
    sq01 = ir.mul(S(0, 1), S(0, 1))
    sq02 = ir.mul(S(0, 2), S(0, 2))
    sq12 = ir.mul(S(1, 2), S(1, 2))
    diagsq = ir.add(ir.add(ir.mul(P00, P00), ir.mul(P11, P11)), ir.mul(P22, P22))
    offsq = ir.add(ir.add(sq01, sq02), sq12)
    p2v = ir.add(diagsq, ir.ts(offsq, 2.0, A_.mult))
    p2c = ir.ts(ir.ts(p2v, 1.0 / 6.0, A_.mult), 1e-30, A_.max)
    pinv = ir.rsqrt_pol(p2c)
    pval = ir.mul(p2c, pinv)

    c0 = ir.sub(ir.mul(P11, P22), sq12)
    c1c = ir.sub(ir.mul(S(0, 1), P22), ir.mul(S(1, 2), S(0, 2)))
    c2c = ir.sub(ir.mul(S(0, 1), S(1, 2)), ir.mul(P11, S(0, 2)))
    detB = ir.add(ir.sub(ir.mul(P00, c0), ir.mul(S(0, 1), c1c)),
                  ir.mul(S(0, 2), c2c))
    pinv3 = ir.mul(ir.mul(pinv, pinv), pinv)
    rr = ir.ts(ir.mul(detB, pinv3), 0.5, A_.mult, 0.9999995, A_.min)
    rr = ir.ts(rr, -0.9999995, A_.max)

    omr = ir.ts(ir.mul(rr, rr), -1.0, A_.mult, 1.0, A_.add)
    rs = ir.rsqrt_pol(omr)
    uu = ir.mul(rr, rs)
    # arctan(u) with range reduction — ACT Arctan domain is [-pi/2, pi/2]:
    # |u|<=1: a = arctan(|u|); |u|>1: pi/2 - arctan(1/|u|); then apply sign.
    au = ir.tt(A_.max, uu, ir.ts(uu, -1.0, A_.mult))      # |u|
    inv = ir.recip(ir.ts(au, 1e-30, A_.max))
    z = ir.tt(A_.min, au, inv)
    az = ir.act(AF.Arctan, z)
    dz = ir.ts(az, -1.0, A_.mult, float(np.pi / 2), A_.add)
    mge = ir.ts(au, 1.0, A_.is_ge)                        # |u| >= 1
    mle = ir.ts(mge, -1.0, A_.mult, 1.0, A_.add)          # 1 - that
    res_abs = ir.add(dz, ir.mul(mle, ir.sub(az, dz)))
    sgn_u = ir.ts(ir.ts(uu, 0.0, A_.is_ge), 2.0, A_.mult, -1.0, A_.add)
    at = ir.mul(res_abs, sgn_u)
    c1t = ir.act(AF.Sin, at, scale=1.0 / 3.0, bias=pi3_ap)
    lam1 = ir.add(q, ir.ts(ir.mul(pval, c1t), 2.0, A_.mult))

    # v1 = best cross of rows of (S - lam1 I)
    D0 = ir.sub(S(0, 0), lam1)
    D1 = ir.sub(S(1, 1), lam1)
    D2 = ir.sub(S(2, 2), lam1)
    rows = [
        [D0, S(0, 1), S(0, 2)],
        [S(0, 1), D1, S(1, 2)],
        [S(0, 2), S(1, 2), D2],
    ]
    best, bn = None, None
    for (i, j) in [(0, 1), (0, 2), (1, 2)]:
        c = ir.cross3(rows[i], rows[j])
        n = ir.dot3(c[0], c[1], c[2], c[0], c[1], c[2])
        if best is None:
            best, bn = c, n
        else:
            m = ir.tt(A_.is_gt, n, bn)
            best = ir.blend3(m, c, best)
            bn = ir.add(bn, ir.mul(m, ir.sub(n, bn)))
    inv = ir.rsqrt_pol(ir.ts(bn, 1e-37, A_.max))
    v1 = [ir.mul(best[0], inv), ir.mul(best[1], inv), ir.mul(best[2], inv)]

    # w2 = best of cross(v1, e_k) (candidates have a zero component)
    zero = ir.ts(v1[0], 0.0, A_.mult)
    nv1 = [ir.ts(v1[i], -1.0, A_.mult) for i in range(3)]
    sqv = [ir.mul(v1[i], v1[i]) for i in range(3)]
    cands = [
        [zero, v1[2], nv1[1]],
        [nv1[2], zero, v1[0]],
        [v1[1], nv1[0], zero],
    ]
    cns = [ir.add(sqv[1], sqv[2]), ir.add(sqv[0], sqv[2]), ir.add(sqv[0], sqv[1])]
    w2b, wn = cands[0], cns[0]
    for k in range(1, 3):
        m = ir.tt(A_.is_gt, cns[k], wn)
        w2b = ir.blend3(m, cands[k], w2b)
        wn = ir.add(wn, ir.mul(m, ir.sub(cns[k], wn)))
    winv = ir.rsqrt_pol(ir.ts(wn, 1e-37, A_.max))
    w2 = [ir.mul(w2b[i], winv) for i in range(3)]
    w3 = ir.cross3(v1, w2)

    def Svec(v):
        return [ir.dot3(S(bi, 0), S(bi, 1), S(bi, 2), v[0], v[1], v[2])
                for bi in range(3)]

    Sw2 = Svec(w2)
    Sw3 = Svec(w3)
    a2x = ir.dot3(w2[0], w2[1], w2[2], Sw2[0], Sw2[1], Sw2[2])
    b2x = ir.dot3(w2[0], w2[1], w2[2], Sw3[0], Sw3[1], Sw3[2])
    c2x = ir.dot3(w3[0], w3[1], w3[2], Sw3[0], Sw3[1], Sw3[2])

    half = ir.ts(ir.sub(a2x, c2x), 0.5, A_.mult)
    mpos = ir.ts(half, 0.0, A_.is_ge)
    sgn = ir.ts(mpos, 2.0, A_.mult, -1.0, A_.add)
    habs = ir.mul(sgn, half)
    rad2 = ir.ts(ir.add(ir.mul(half, half), ir.mul(b2x, b2x)), 1e-37, A_.max)
    radi = ir.rsqrt_pol(rad2)
    rad = ir.mul(rad2, radi)
    pos = ir.ts(ir.add(habs, rad), 1e-37, A_.max)
    tq = ir.mul(ir.mul(b2x, ir.recip(pos)), sgn)
    c2i = ir.rsqrt_pol(ir.ts(ir.mul(tq, tq), 1.0, A_.add))
    s2i = ir.mul(tq, c2i)
    tb = ir.mul(tq, b2x)
    lamA = ir.add(a2x, tb)
    lamB = ir.sub(c2x, tb)
    mAB = ir.tt(A_.is_ge, lamA, lamB)
    vA = [ir.add(ir.mul(c2i, w2[i]), ir.mul(s2i, w3[i])) for i in range(3)]
    vB = [ir.sub(ir.mul(c2i, w3[i]), ir.mul(s2i, w2[i])) for i in range(3)]
    v2 = ir.blend3(mAB, vA, vB)
    v3 = ir.cross3(v1, v2)

    def Avec(v):
        return [ir.dot3(Ae[ai][0], Ae[ai][1], Ae[ai][2], v[0], v[1], v[2])
                for ai in range(3)]

    b1 = Avec(v1)
    n1 = ir.dot3(b1[0], b1[1], b1[2], b1[0], b1[1], b1[2])
    i1 = ir.rsqrt_pol(ir.ts(n1, 1e-37, A_.max))
    u1 = [ir.mul(b1[i], i1) for i in range(3)]

    b2v = Avec(v2)
    dd = ir.dot3(u1[0], u1[1], u1[2], b2v[0], b2v[1], b2v[2])
    b2o = [ir.sub(b2v[i], ir.mul(dd, u1[i])) for i in range(3)]
    n2 = ir.dot3(b2o[0], b2o[1], b2o[2], b2o[0], b2o[1], b2o[2])
    i2 = ir.rsqrt_pol(ir.ts(n2, 1e-37, A_.max))
    u2 = [ir.mul(b2o[i], i2) for i in range(3)]

    u3 = ir.cross3(u1, u2)

    us = [u1, u2, u3]
    vs = [v1, v2, v3]
    for ai in range(3):
        for bi in range(3):
            t1 = ir.mul(us[0][ai], vs[0][bi])
            t2 = ir.mul(us[1][ai], vs[1][bi])
            sgm = ir.add(t1, t2)
            t3 = ir.mul(us[2][ai], vs[2][bi])
            ir.add(sgm, t3, out=Rv[:, 3 * ai + bi, :])

    mn = [ir.ts(me[i], -1.0, A_.mult) for i in range(3)]
    for bi in range(3):
        t1 = ir.mul(mn[0], Rv[:, bi, :])
        t2 = ir.mul(mn[1], Rv[:, 3 + bi, :])
        sgm = ir.add(t1, t2)
        t3 = ir.mul(mn[2], Rv[:, 6 + bi, :])
        ir.add(sgm, t3, out=Rv[:, 9 + bi, :])


# ----------------------------------------------------------------------------
# Bass program
# ----------------------------------------------------------------------------

def _split_multiwait(nc):
    """This walrus build encodes at most ONE semaphore wait per instruction,
    but Tile emits several. Split extras into standalone EventSemaphore
    (pure wait) instructions on the same engine, immediately before."""
    from concourse import mybir
    import bass_rust

    n_split = 0
    for fn in nc.m.functions:
        for blk in fn.blocks:
            new = []
            for ins in blk.instructions:
                si = ins.sync_info
                if si is not None and si.on_wait is not None and len(si.on_wait) > 1:
                    waits = list(si.on_wait)
                    for k, w in enumerate(waits[:-1]):
                        new.append(mybir.InstEventSemaphore(
                            name=f"{ins.name}-w{k}",
                            engine=ins.engine,
                            sync_info=bass_rust.SyncInfo(
                                on_wait=[w], on_update=[]),
                        ))
                        n_split += 1
                    ins.sync_info = bass_rust.SyncInfo(
                        on_wait=[waits[-1]],
                        on_update=list(si.on_update or []))
                new.append(ins)
            blk.instructions = new
    return n_split


def _build_program(ls=LS, n_slots=56, split_waits=True, prefetch=8):
    import concourse.bass as bass
    import concourse.tile as tile
    from concourse import mybir

    f32 = mybir.dt.float32
    bf16 = mybir.dt.bfloat16
    A_ = mybir.AluOpType
    AF = mybir.ActivationFunctionType

    nt = ls // 128
    C = nt                       # math column count (one col per 128-frame group)

    nc = bass.Bass("TRN2", target_bir_lowering=False, debug=False)

    xgt_d = nc.dram_tensor("xgt", [192, ls], bf16, kind="ExternalInput").ap()
    xsep_d = nc.dram_tensor("xsep", [ls, 768], bf16, kind="ExternalInput").ap()
    w_d = nc.dram_tensor("wm", [192, 12], bf16, kind="ExternalInput").ap()
    out_d = nc.dram_tensor("out", [ls, 768], bf16, kind="ExternalOutput").ap()

    with tile.TileContext(nc) as tc:
        with (
            tc.tile_pool(name="wp", bufs=1) as wp,
            tc.tile_pool(name="gp_", bufs=2) as gpool,
            tc.tile_pool(name="ep", bufs=1) as ep,
            tc.tile_pool(name="ps", bufs=3, space="PSUM") as psp,
            tc.tile_pool(name="ps2", bufs=1, space="PSUM") as pss,
            tc.tile_pool(name="xp", bufs=prefetch) as xp,
            tc.tile_pool(name="p2", bufs=6) as p2p,
            tc.tile_pool(name="op_", bufs=3) as opool,
        ):
            # ---------------- constants / weights ----------------
            w0 = wp.tile([128, 12], bf16, tag="w0")
            w1 = wp.tile([64, 12], bf16, tag="w1")
            nc.sync.dma_start(w0[:], w_d[0:128, :])
            nc.sync.dma_start(w1[:], w_d[128:192, :])

            E = ep.tile([128, nt * 12], f32, tag="E")
            R = ep.tile([128, nt * 12], f32, tag="R")
            MS = ep.tile([128, n_slots * C], f32, tag="MS")
            pi3 = ep.tile([128, 1], f32, tag="pi3")
            nc.gpsimd.memset(pi3[:], float(np.pi / 3))
            Ev = E[:].rearrange("p (g e) -> p e g", e=12)
            Rv = R[:].rearrange("p (g e) -> p e g", e=12)

            # ---------------- phase 1: E = xgT^T @ W ----------------
            # The Matmult ISA slot encodes at most ONE semaphore wait, so each
            # real matmul must need at most one fresh semaphore. Dummy PE
            # matmuls "absorb" each DMA's semaphore into the PE's observed
            # clock first (engine-internal ordering then needs no sems).
            ps_scr = pss.tile([128, 12], f32, tag="scr")
            nc.tensor.matmul(ps_scr[0:12, 0:12], w0[:, 0:12], w0[:],
                             start=True, stop=True)
            nc.tensor.matmul(ps_scr[0:12, 0:12], w1[:, 0:12], w1[:],
                             start=True, stop=True)
            n_slab = nt // 16
            for s in range(n_slab):
                sl0 = gpool.tile([128, 2048], bf16, tag="g0")
                sl1 = gpool.tile([64, 2048], bf16, tag="g1")
                nc.sync.dma_start(sl0[:], xgt_d[0:128, s * 2048:(s + 1) * 2048])
                nc.sync.dma_start(sl1[:], xgt_d[128:192, s * 2048:(s + 1) * 2048])
                nc.tensor.matmul(ps_scr[0:12, 0:12], sl0[:, 0:12], sl0[:, 0:12],
                                 start=True, stop=True)
                nc.tensor.matmul(ps_scr[0:12, 0:12], sl1[:, 0:12], sl1[:, 0:12],
                                 start=True, stop=True)
                psE = psp.tile([128, 192], f32, tag="psE")
                for g in range(16):
                    nc.tensor.matmul(psE[:, g * 12:(g + 1) * 12],
                                     sl0[:, g * 128:(g + 1) * 128], w0[:],
                                     start=True, stop=False)
                    nc.tensor.matmul(psE[:, g * 12:(g + 1) * 12],
                                     sl1[:, g * 128:(g + 1) * 128], w1[:],
                                     start=False, stop=True)
                nc.scalar.copy(E[:, s * 192:(s + 1) * 192], psE[:])

            # ---------------- phase 2: rotation math (single C=64 pass) ---
            ir = _MathIR(A_)
            _record_math(ir, Ev, Rv, pi3[:])
            _emit_math(nc, ir, MS[:], C, n_slots)

            # ---------------- phase 3: apply (bf16, vector-only chains) ---
            # Prefetch all xq loads ahead of the out DMAs in the SP queue;
            # the xp pool depth gates them at runtime (self-balancing).
            n_grp = nt // 4
            xqs = []
            for grp in range(n_grp):
                xq = xp.tile([128, 4 * 768], bf16, tag="xq")
                src = xsep_d[grp * 512:(grp + 1) * 512, :].rearrange(
                    "(g p) c -> p g c", p=128)
                nc.sync.dma_start(xq[:].rearrange("p (g c) -> p g c", c=768), src)
                xqs.append(xq)
            for grp in range(n_grp):
                xq = xqs[grp]
                for t in range(4):
                    gg = grp * 4 + t
                    base = t * 768
                    if t % 2 == 0:
                        ot = opool.tile([128, 2 * 768], bf16, tag="ot")
                    obase = (t % 2) * 768
                    # per component b: u = x0*R0b + tneg_b (ACT MAD);
                    # a = x1*R1b + u (DVE STT); ot_b = x2*R2b + a (DVE STT).
                    for bi in range(3):
                        rcol0 = R[:, gg * 12 + bi: gg * 12 + bi + 1]
                        rcol1 = R[:, gg * 12 + 3 + bi: gg * 12 + 3 + bi + 1]
                        rcol2 = R[:, gg * 12 + 6 + bi: gg * 12 + 6 + bi + 1]
                        tncol = R[:, gg * 12 + 9 + bi: gg * 12 + 9 + bi + 1]
                        x0 = xq[:, base:base + 256]
                        x1 = xq[:, base + 256:base + 512]
                        x2 = xq[:, base + 512:base + 768]
                        u_t = p2p.tile([128, 256], bf16, tag=f"u{bi}")
                        a_t = p2p.tile([128, 256], bf16, tag=f"a{bi}")
                        nc.scalar.activation(u_t[:], x0, AF.Identity,
                                             bias=tncol, scale=rcol0)
                        nc.vector.scalar_tensor_tensor(
                            a_t[:], x1, rcol1, u_t[:], A_.mult, A_.add)
                        nc.vector.scalar_tensor_tensor(
                            ot[:, obase + bi * 256:obase + (bi + 1) * 256],
                            x2, rcol2, a_t[:], A_.mult, A_.add)
                    if t % 2 == 1:
                        dst = out_d[(gg - 1) * 128:(gg + 1) * 128, :].rearrange(
                            "(g p) c -> p g c", p=128)
                        nc.sync.dma_start(dst, ot[:].rearrange(
                            "p (g c) -> p g c", c=768))

    if split_waits:
        _split_multiwait(nc)
    return nc


# ----------------------------------------------------------------------------
# Host-side preparation
# ----------------------------------------------------------------------------

def _prep_inputs(x, ref_x, align_idx):
    import ml_dtypes
    BF16 = ml_dtypes.bfloat16
    x = np.asarray(x, dtype=F32)
    ref_x = np.asarray(ref_x)
    idx = np.asarray(align_idx).astype(np.int64)
    L = x.shape[0]

    ref64 = ref_x.astype(np.float64)
    ref_c = (ref64 - ref64.mean(0)).astype(F32)        # [64, 3]

    xg = x[:, idx, :]                                   # [L, 64, 3]
    xgt = np.ascontiguousarray(xg.reshape(L, 192).T).astype(BF16)

    xsep = np.ascontiguousarray(
        x.transpose(0, 2, 1)).reshape(L, 768).astype(BF16)

    W = np.zeros((192, 12), dtype=F32)
    for a in range(3):
        rows = 3 * np.arange(N_ALIGN) + a
        for b in range(3):
            W[rows, 3 * a + b] = ref_c[:, b]
        W[rows, 9 + a] = F32(1.0 / N_ALIGN)
    return xgt, xsep, W.astype(BF16)


# ----------------------------------------------------------------------------
# Runner: jit once, reuse
# ----------------------------------------------------------------------------

class _Runner:
    def __init__(self):
        import jax

        self.jax = jax
        self.nc = _build_program(LS)
        self._build_exec()

    def _build_exec(self):
        import jax
        from jax.sharding import Mesh, PartitionSpec
        from jax.experimental.shard_map import shard_map
        from concourse import mybir
        from concourse.bass2jax import (_bass_exec_p, install_neuronx_cc_hook,
                                        partition_id_tensor)

        install_neuronx_cc_hook()
        # surface compile-hook exceptions (PJRT swallows them)
        try:
            import libneuronxla
            import traceback
            if not getattr(libneuronxla, "_ant_logged_cc", False):
                _orig_cc = libneuronxla.neuronx_cc

                def _logged_cc(*a, **k):
                    try:
                        return _orig_cc(*a, **k)
                    except BaseException:
                        traceback.print_exc()
                        raise

                libneuronxla.neuronx_cc = _logged_cc
                libneuronxla._ant_logged_cc = True
        except ImportError:
            pass
        nc = self.nc

        part_name = (nc.partition_id_tensor.name
                     if nc.partition_id_tensor else None)
        in_names, out_names, out_avals = [], [], []
        for alloc in nc.m.functions[0].allocations:
            if not isinstance(alloc, mybir.MemoryLocationSet):
                continue
            name = alloc.memorylocations[0].name
            if alloc.kind == "ExternalInput":
                if name != part_name:
                    in_names.append(name)
            elif alloc.kind == "ExternalOutput":
                shape = tuple(alloc.tensor_shape)
                dtype = mybir.dt.np(alloc.dtype)
                out_names.append(name)
                out_avals.append(jax.core.ShapedArray(shape, dtype))
        self.in_names = list(in_names)
        self.out_names = list(out_names)
        n_params = len(in_names)
        all_names = in_names + out_names
        if part_name is not None:
            all_names = all_names + [part_name]

        def _body(*args):
            operands = list(args)
            if part_name is not None:
                operands.append(partition_id_tensor())
            outs = _bass_exec_p.bind(
                *operands,
                out_avals=tuple(out_avals),
                in_names=tuple(all_names),
                out_names=tuple(out_names),
                lowering_input_output_aliases=(),
                sim_require_finite=True,
                sim_require_nnan=True,
                nc=nc,
            )
            return tuple(outs)

        devices = jax.devices()[:N_CORES]
        mesh = Mesh(np.asarray(devices), ("core",))
        n_outs = len(out_names)
        in_specs = (PartitionSpec("core"),) * (n_params + n_outs)
        out_specs = (PartitionSpec("core"),) * n_outs
        self._fn = jax.jit(
            shard_map(_body, mesh=mesh, in_specs=in_specs,
                      out_specs=out_specs, check_rep=False),
            keep_unused=True,
        )
        self._zeros = [
            np.zeros((N_CORES * av.shape[0], *av.shape[1:]), av.dtype)
            for av in out_avals
        ]

    def stage(self, x, ref_x, align_idx):
        xgt, xsep, W = _prep_inputs(x, ref_x, align_idx)
        per_name = {
            "xgt": np.concatenate(
                [xgt[:, c * LS:(c + 1) * LS] for c in range(N_CORES)], axis=0),
            "xsep": xsep,
            "wm": np.concatenate([W] * N_CORES, axis=0),
        }
        args = [per_name[n] for n in self.in_names] + list(self._zeros)
        return [self.jax.device_put(a) for a in args]

    def run_staged(self, staged):
        return self._fn(*staged)

    def run(self, x, ref_x, align_idx):
        staged = self.stage(x, ref_x, align_idx)
        outs = self.run_staged(staged)
        out = np.asarray(outs[self.out_names.index("out")]).astype(np.float32)
        L = out.shape[0]
        return np.ascontiguousarray(
            out.reshape(L, 3, N_INP).transpose(0, 2, 1))


def _get_runner():
    global _RUNNER
    if _RUNNER is None:
        _RUNNER = _Runner()
    return _RUNNER


def kernel(x, ref_x, align_idx):
    runner = _get_runner()
    return runner.run(x, ref_x, align_idx).astype(np.float32)


if __name__ == "__main__":
    nc = _build_program(LS)
    print("built ok")

